# revision 2
# baseline (speedup 1.0000x reference)
"""Bidirectional GRU (shared weights) on 8 NeuronCores, data-parallel over batch.

Shapes: x [4096,16,1024], h0/bi_h0 [4096,1024] -> out [4096,16,2048].
Per core: batch shard of 512.

v5 scheme (evolves v4; v4 measured 1.80ms vs v2's 2.15ms):
  * t-gate RECURRENT matmul moves to fp8 DoubleRow (numerics: 1.63e-2
    relmax vs the 2e-2 gate; the numpy model has matched HW relmax to
    1e-5 twice).  20 -> 16 matmuls per block.
  * bf16 elementwise intermediates (rz, u, w, htl, dif, m): the all-16-
    bit tail ops run ~2x on DVE, so DVE absorbs m/hn and GP keeps only
    w/dif -- every engine back under the shrunken PE block time.
  r-gate matmuls  : fp8 e4m3 DoubleRow (weights x16)
  z/t-gate matmuls: bf16 (fp8 anywhere else blows the 2e-2 relmax gate --
                    measured in numerics.py: xproj-z-fp8 2.06e-2,
                    rec-z-fp8 2.24e-2, gx-fp8 2.10e-2)
  * Unified x16 scale: ALL weights (fp8 AND bf16 -- exact in bf16) are
    stored pre-scaled x16, so every PSUM result and every gx slice is
    x16; the phase-1 biases (x16) are folded into gx at evacuation.
    Phase-2 preactivation math is then pure adds and the 1/16 descale
    rides the ACT sigmoid/tanh `scale`.
  * out_T is bf16: the blended h IS both the stored output and the
    carried bf16 h -- kills the separate f32 out store and the bf16
    CAST that made gpsimd the phase-2 bottleneck (94% busy in v2).
  * Phase-2 loop is d-OUTER (8 j-blocks of dir 0, then dir 1): a step's
    last h-tile has ~8 blocks of slack before the next same-direction
    step consumes it (v2's d-inner gave ~1 block => ~3us PE stall per
    step; measured 162us of phase-2 PE gaps).
  * gx layout [T, 24, 128, BS]: slots 2j/2j+1 hold the r_j/z_j pair
    (one DMA load, one combined [128,2,BS] sigmoid), slots 16+j = t_j.
  * Engine split: DVE = preacts (zpre/rpre/u/w), ACT = rz-sigmoid/tanh
    (+ fp8 h shadow cast), GP = pure tail (dif/m/blend) -- each engine
    under the ~4.5us PE block time, with clean per-engine FIFO order.
"""
import re
from contextlib import ExitStack

import numpy as np

import bass_rust
import concourse.bass as bass
import concourse.mybir as mybir
import concourse.tile as tile
from concourse.bass_utils import run_bass_kernel_spmd


class ChunkedDrainTileContext(tile.TileContext):
    """TileContext whose kernel-tail drain is split into one drain instruction
    per semaphore — this walrus build rejects a sync CTRL instruction with
    more than one sync wait ("Too many sync wait commands")."""

    def _drain_and_barrier(self, tick_clock, wait_clock):
        gc = tick_clock.global_clock
        vals = [int(v) for v in re.findall(r"\d+", repr(gc))]
        for p, v in enumerate(vals):
            if v <= 0:
                continue
            vc = bass_rust.VectorClock()
            vc.require_at_least(p, v)
            drain_inst = self.nc.sync.drain()
            wait_clock.add_sem_waits(drain_inst.ins, bass_rust.ScopedClock({None: vc}))

        self.nc.all_engine_barrier()
        assert self.sems is not None
        popped = self.nc._tile_sem_poison_stack.pop()
        assert popped is self._sem_poison
        self.nc.clear_and_free_semaphores(list(self.sems.allocated().values()))
        self.nc.all_engine_barrier()


def _legalize_single_wait(nc):
    """This walrus build rejects any instruction carrying more than one sync
    wait.  Hoist all but the last wait of each instruction onto freshly
    inserted same-engine NoOps placed immediately before it."""
    ctr = 0
    for fn in nc.m.functions:
        for bb in fn.blocks:
            insts = bb.instructions
            if not any(
                i.sync_info and i.sync_info.on_wait and len(i.sync_info.on_wait) > 1
                for i in insts
            ):
                continue
            out = []
            for inst in list(insts):
                si = inst.sync_info
                if si is not None and si.on_wait and len(si.on_wait) > 1:
                    SI = type(si)
                    waits = list(si.on_wait)
                    for w in waits[:-1]:
                        ctr += 1
                        nop = mybir.InstNoOp(name=f"waitnop_{ctr}")
                        nop.engine = inst.engine
                        nop.sync_info = SI(on_wait=[w], on_update=[])
                        out.append(nop)
                    inst.sync_info = SI(
                        on_wait=[waits[-1]], on_update=list(si.on_update or [])
                    )
                out.append(inst)
            insts.clear()
            insts.extend(out)
    return ctr


B, T, I, H = 4096, 16, 1024, 1024
NCORES = 8
BS = B // NCORES  # 512
F32 = mybir.dt.float32
BF16 = mybir.dt.bfloat16
FP8 = mybir.dt.float8e4
ACT = mybir.ActivationFunctionType
ALU = mybir.AluOpType
DR = mybir.MatmulPerfMode.DoubleRow

KT = I // 128       # 8 contraction tiles of 128
NJ = H // 128       # 8 h-dim tiles
WSCALE = 16.0       # all weights stored x16; descale rides ACT scale
INV = 1.0 / WSCALE


def build(t_steps: int = T):
    nc = bass.Bass("TRN2", num_devices=NCORES)

    x8T = nc.declare_dram_parameter("x8T", [T, I, BS], FP8, isOutput=False)
    xbT = nc.declare_dram_parameter("xbT", [T, I, BS], BF16, isOutput=False)
    h0T8 = nc.declare_dram_parameter("h0T8", [H, BS], FP8, isOutput=False)
    h0Tb = nc.declare_dram_parameter("h0Tb", [H, BS], BF16, isOutput=False)
    bi_h0T8 = nc.declare_dram_parameter("bi_h0T8", [H, BS], FP8, isOutput=False)
    bi_h0Tb = nc.declare_dram_parameter("bi_h0Tb", [H, BS], BF16, isOutput=False)
    Wx8 = nc.declare_dram_parameter("Wx8", [I, H], FP8, isOutput=False)
    Wxb = nc.declare_dram_parameter("Wxb", [I, 2 * H], BF16, isOutput=False)
    Wh8 = nc.declare_dram_parameter("Wh8", [H, 2 * H], FP8, isOutput=False)
    Whb = nc.declare_dram_parameter("Whb", [H, H], BF16, isOutput=False)
    bx16 = nc.declare_dram_parameter("bx16", [3 * H], F32, isOutput=False)
    bht16 = nc.declare_dram_parameter("bht16", [H], F32, isOutput=False)
    out_T = nc.declare_dram_parameter("out_T", [t_steps, 2 * H, BS], BF16,
                                      isOutput=True)

    with ChunkedDrainTileContext(nc) as tc, ExitStack() as ctx:
        singles = ctx.enter_context(tc.tile_pool(name="singles", bufs=1))
        # phase-1 evac biases (x16), one [128,1] column per og slice
        bx_sb = singles.tile([128, 24], F32)
        nc.sync.dma_start(out=bx_sb, in_=bx16.rearrange("(o p) -> p o", p=128))
        bht_sb = singles.tile([128, NJ], F32)
        nc.sync.dma_start(out=bht_sb, in_=bht16.rearrange("(o p) -> p o", p=128))

        # gx scratch: slot 2j / 2j+1 = r_j / z_j, slot 16+j = t_j  (all x16,
        # bx biases folded; b_ht NOT folded -- it multiplies r first)
        gx_pool = ctx.enter_context(tc.tile_pool(name="gxdram", bufs=1, space="DRAM"))
        gx = gx_pool.tile([t_steps, 24, 128, BS], BF16)

        # Weight slot pools shared by both phases; phase-2 tiles rotate into
        # phase-1's slots so each Wh DMA starts as soon as the matching Wx
        # slice's last phase-1 reader is done.
        w8pool = ctx.enter_context(tc.tile_pool(name="w8pool", bufs=3 * NJ))
        wbpool = ctx.enter_context(tc.tile_pool(name="wbpool", bufs=2 * NJ))

        def load_w8(src, c, nm, split=False):
            wt = w8pool.tile([128, KT, 128], FP8, name=f"{nm}{c}", tag="w8")
            wr = src[:, c * 128:(c + 1) * 128].rearrange("(k p) o -> p k o", p=128)
            if split:  # startup: spread across queues so PE starts early
                for k in range(KT):
                    nc.sync.dma_start(out=wt[:, k, :], in_=wr[:, k, :])
            else:
                nc.sync.dma_start(out=wt, in_=wr)
            return wt

        def load_wb(src, c, nm):
            wt = wbpool.tile([128, KT, 128], BF16, name=f"{nm}{c}", tag="wb")
            wr = src[:, c * 128:(c + 1) * 128].rearrange("(k p) o -> p k o", p=128)
            nc.sync.dma_start(out=wt, in_=wr)
            return wt

        # first r og slice + first x tile first, so PE starts a few us in
        wx8 = [load_w8(Wx8, 0, "wx8", split=True)]

        h8pool = ctx.enter_context(tc.tile_pool(name="h8pool", bufs=4))
        hbpool = ctx.enter_context(tc.tile_pool(name="hbpool", bufs=4))

        # ---- phase 1: x projections for all timesteps -> gx (bf16) ----
        p1 = ExitStack()
        p1_x8 = p1.enter_context(tc.tile_pool(name="p1x8", bufs=2))
        p1_xb = p1.enter_context(tc.tile_pool(name="p1xb", bufs=2))
        p1_ev = p1.enter_context(tc.tile_pool(name="p1ev", bufs=6))
        p1_ps = p1.enter_context(tc.tile_pool(name="p1ps", bufs=8, space="PSUM"))

        def load_xt(t):
            x8_sb = p1_x8.tile([128, KT, BS], FP8, name=f"x8_{t}", tag="x8")
            xb_sb = p1_xb.tile([128, KT, BS], BF16, name=f"xb_{t}", tag="xb")
            x8r = x8T[t].rearrange("(k p) b -> p k b", p=128)
            xbr = xbT[t].rearrange("(k p) b -> p k b", p=128)
            last = None
            for k in range(KT):
                nc.sync.dma_start(out=x8_sb[:, k, :], in_=x8r[:, k, :])
                last = nc.sync.dma_start(out=xb_sb[:, k, :], in_=xbr[:, k, :])
            return x8_sb, xb_sb, last

        h_cur8 = {}
        h_curb = {}
        xt_next = load_xt(0)
        h_gate = None  # h-state loads wait on this so they can't steal
        for t in range(t_steps):  # startup DMA bandwidth
            x8_sb, xb_sb, _ = xt_next
            if t == 0:
                for j in range(1, NJ):
                    wx8.append(load_w8(Wx8, j, "wx8"))
                wxb = [load_wb(Wxb, c, "wxb") for c in range(2 * NJ)]
            if t + 1 < t_steps:
                xt_next = load_xt(t + 1)
                if t + 1 == 2:
                    h_gate = xt_next[2]
            if t == min(3, t_steps - 1):
                # initial hidden states (fp8 + bf16): off critical path
                for d in (0, 1):
                    h_cur8[d] = h8pool.tile(
                        [128, NJ, BS], FP8, name=f"h8_d{d}_init", tag="h8")
                    h_curb[d] = hbpool.tile(
                        [128, NJ, BS], BF16, name=f"hb_d{d}_init", tag="hb")
                    src8 = h0T8 if d == 0 else bi_h0T8
                    srcb = h0Tb if d == 0 else bi_h0Tb
                    for dst, src in ((h_cur8[d], src8), (h_curb[d], srcb)):
                        hdma = nc.gpsimd.dma_start(
                            out=dst,
                            in_=src.rearrange("(j p) b -> p j b", p=128),
                        )
                        if h_gate is not None:
                            tile.add_dep_helper(
                                hdma.ins, h_gate.ins, sync=True,
                                reason="h loads after startup-critical streams")
            for og in range(24):  # 0..7 r, 8..15 z, 16..23 t
                ps = p1_ps.tile([128, BS], F32, name=f"p1ps{t}_{og}", tag="ps1")
                if og < NJ:  # r gate: fp8 DoubleRow over k-pairs
                    for kp in range(KT // 2):
                        nc.tensor.matmul(
                            ps,
                            wx8[og][:, 2 * kp:2 * kp + 2, :],
                            x8_sb[:, 2 * kp:2 * kp + 2, :],
                            start=(kp == 0),
                            stop=(kp == KT // 2 - 1),
                            perf_mode=DR,
                        )
                else:  # z/t gates: bf16 (weights x16)
                    for k in range(KT):
                        nc.tensor.matmul(
                            ps,
                            wxb[og - NJ][:, k, :],
                            xb_sb[:, k, :],
                            start=(k == 0),
                            stop=(k == KT - 1),
                        )
                ev = p1_ev.tile([128, BS], BF16, name=f"p1ev{t}_{og}", tag="ev")
                nc.scalar.activation(ev, ps, ACT.Identity, bias=bx_sb[:, og:og + 1])
                # gx slot: r_j -> 2j, z_j -> 2j+1, t_j -> 16+j
                j = og % NJ
                slot = 2 * j if og < NJ else (2 * j + 1 if og < 2 * NJ else 16 + j)
                # store via Pool: these wait on ACT and would head-of-line
                # block later x/weight loads on SP
                nc.gpsimd.dma_start(out=gx[t, slot], in_=ev)

        p1.close()

        # ---- phase 2: both scans, d outer within each step ----
        wh8 = [load_w8(Wh8, c, "wh8") for c in range(2 * NJ)]  # r j<8, t j>=8
        whb = [load_wb(Whb, c, "whb") for c in range(NJ)]    # z

        gx_sb_pool = ctx.enter_context(tc.tile_pool(name="gxsb", bufs=8))
        rz_pool = ctx.enter_context(tc.tile_pool(name="rz", bufs=6))
        tmp_pool = ctx.enter_context(tc.tile_pool(name="tmp", bufs=10))
        p2_ps = ctx.enter_context(tc.tile_pool(name="p2ps", bufs=8, space="PSUM"))

        # post-tanh tail of block b is emitted after block b+1's
        # preactivation ops; GP is a pure tail engine so its FIFO never
        # head-of-line-blocks preactivation work
        pending = []

        def emit_tail(htl, z_ap, hcb, hn8, hnb, j, d, ot, sfx):
            dif = tmp_pool.tile([128, BS], BF16, name=f"dif{sfx}", tag="tm")
            nc.gpsimd.tensor_sub(dif, htl, hcb[:, j, :])
            m_sb = tmp_pool.tile([128, BS], BF16, name=f"m{sfx}", tag="tm")
            nc.vector.tensor_mul(m_sb, z_ap, dif)
            # blended h in bf16: carried state AND the stored output
            nc.vector.tensor_add(hnb[:, j, :], hcb[:, j, :], m_sb)
            off = 0 if d == 0 else H
            nc.sync.dma_start(
                out=out_T[ot, off + j * 128:off + (j + 1) * 128, :],
                in_=hnb[:, j, :],
            )
            nc.scalar.activation(hn8[:, j, :], hnb[:, j, :], ACT.Identity)

        for t in range(t_steps):
            h_new8 = {
                d: h8pool.tile([128, NJ, BS], FP8, name=f"h8_d{d}_t{t}", tag="h8")
                for d in (0, 1)
            }
            h_newb = {
                d: hbpool.tile([128, NJ, BS], BF16, name=f"hb_d{d}_t{t}", tag="hb")
                for d in (0, 1)
            }
            for d in (0, 1):
                tt = t if d == 0 else t_steps - 1 - t   # gx slice this step
                ot = t_steps - 1 - t if d == 0 else t   # output time slot
                hc8, hcb = h_cur8[d], h_curb[d]
                for j in range(NJ):
                    sfx = f"{d}_{t}_{j}"

                    # z group first: it consumes the bf16 h whose producing
                    # chain is one op shorter than the fp8 shadow's
                    z_ps = p2_ps.tile([128, BS], F32, name=f"psz{sfx}", tag="ps2")
                    for k in range(KT):
                        nc.tensor.matmul(
                            z_ps, whb[j][:, k, :], hcb[:, k, :],
                            start=(k == 0), stop=(k == KT - 1),
                        )
                    r_ps = p2_ps.tile([128, BS], F32, name=f"psr{sfx}", tag="ps2")
                    for kp in range(KT // 2):
                        nc.tensor.matmul(
                            r_ps,
                            wh8[j][:, 2 * kp:2 * kp + 2, :],
                            hc8[:, 2 * kp:2 * kp + 2, :],
                            start=(kp == 0), stop=(kp == KT // 2 - 1),
                            perf_mode=DR,
                        )
                    t_ps = p2_ps.tile([128, BS], F32, name=f"pst{sfx}", tag="ps2")
                    for kp in range(KT // 2):
                        nc.tensor.matmul(
                            t_ps,
                            wh8[NJ + j][:, 2 * kp:2 * kp + 2, :],
                            hc8[:, 2 * kp:2 * kp + 2, :],
                            start=(kp == 0), stop=(kp == KT // 2 - 1),
                            perf_mode=DR,
                        )

                    # one DMA for the r/z pair, one for t
                    gxrz = gx_sb_pool.tile(
                        [128, 2, BS], BF16, name=f"gxrz{sfx}", tag="gx")
                    nc.sync.dma_start(
                        out=gxrz,
                        in_=gx[tt, 2 * j:2 * j + 2].rearrange("s p b -> p s b"))
                    gxt = gx_sb_pool.tile([128, BS], BF16, name=f"gxt{sfx}", tag="gx")
                    nc.sync.dma_start(out=gxt, in_=gx[tt, 16 + j])

                    # preactivations (everything x16; bx biases already in gx)
                    rzpre = tmp_pool.tile(
                        [128, 2, BS], F32, name=f"rzpre{sfx}", tag="tm2")
                    nc.vector.tensor_add(rzpre[:, 1, :], z_ps, gxrz[:, 1, :])
                    nc.vector.tensor_add(rzpre[:, 0, :], r_ps, gxrz[:, 0, :])
                    rz_sb = rz_pool.tile([128, 2, BS], BF16, name=f"rz{sfx}", tag="rz")
                    nc.scalar.activation(rz_sb, rzpre, ACT.Sigmoid, scale=INV)

                    # u = (t_ps + 16*b_ht) * r ; w = u + gxt ; htl = tanh(w/16)
                    u_sb = tmp_pool.tile([128, BS], BF16, name=f"u{sfx}", tag="tm")
                    nc.vector.scalar_tensor_tensor(
                        u_sb, t_ps, bht_sb[:, j:j + 1], rz_sb[:, 0, :],
                        op0=ALU.add, op1=ALU.mult)
                    w_sb = tmp_pool.tile([128, BS], BF16, name=f"w{sfx}", tag="tm")
                    nc.gpsimd.tensor_add(w_sb, u_sb, gxt)
                    htl = tmp_pool.tile([128, BS], BF16, name=f"htl{sfx}", tag="tm")
                    nc.scalar.activation(htl, w_sb, ACT.Tanh, scale=INV)

                    # tail of the previous block AFTER this block's preacts:
                    # keeps hn8(b-1) behind rz/tanh(b) in ACT's FIFO, else a
                    # serial cross-block cycle forms through the GP tail
                    if pending:
                        emit_tail(*pending.pop())

                    pending.append(
                        (htl, rz_sb[:, 1, :], hcb, h_new8[d], h_newb[d],
                         j, d, ot, sfx))
            h_cur8 = h_new8
            h_curb = h_newb
        if pending:
            emit_tail(*pending.pop())

    _legalize_single_wait(nc)
    return nc


_built = {}


def _get_nc(t_steps: int = T):
    if t_steps not in _built:
        _built[t_steps] = build(t_steps)
    return _built[t_steps]


def prep_in_maps(x, h0, bi_h0, W_izr, b_izr, W_hzr, b_hzr, W_it, b_it, W_ht, b_ht):
    import ml_dtypes

    FP8NP = ml_dtypes.float8_e4m3
    BF16NP = ml_dtypes.bfloat16

    xT = x.reshape(NCORES, BS, T, I).transpose(0, 2, 3, 1)  # [NC, T, I, BS]
    x8T = np.ascontiguousarray(xT.astype(FP8NP))
    xbT = np.ascontiguousarray(xT.astype(BF16NP))
    h0T = h0.reshape(NCORES, BS, H).transpose(0, 2, 1)
    bi_h0T = bi_h0.reshape(NCORES, BS, H).transpose(0, 2, 1)
    # all weights stored x16 (exact in bf16; fp8 needs it vs subnormals)
    Wx8 = np.ascontiguousarray((W_izr[:H].T * WSCALE).astype(FP8NP))
    Wxb = np.ascontiguousarray(
        (np.concatenate([W_izr[H:].T, W_it.T], axis=1) * WSCALE).astype(BF16NP))
    Wh8 = np.ascontiguousarray(
        (np.concatenate([W_hzr[:H].T, W_ht.T], axis=1) * WSCALE).astype(FP8NP))
    Whb = np.ascontiguousarray((W_hzr[H:].T * WSCALE).astype(BF16NP))
    bx16 = np.ascontiguousarray(
        (np.concatenate([b_izr + b_hzr, b_it]) * WSCALE).astype(np.float32))
    bht16 = np.ascontiguousarray((b_ht * WSCALE).astype(np.float32))
    return [
        {
            "x8T": x8T[c],
            "xbT": xbT[c],
            "h0T8": np.ascontiguousarray(h0T[c].astype(FP8NP)),
            "h0Tb": np.ascontiguousarray(h0T[c].astype(BF16NP)),
            "bi_h0T8": np.ascontiguousarray(bi_h0T[c].astype(FP8NP)),
            "bi_h0Tb": np.ascontiguousarray(bi_h0T[c].astype(BF16NP)),
            "Wx8": Wx8,
            "Wxb": Wxb,
            "Wh8": Wh8,
            "Whb": Whb,
            "bx16": bx16,
            "bht16": bht16,
        }
        for c in range(NCORES)
    ]


def kernel(**inputs):
    inputs = {k: np.asarray(v, dtype=np.float32) for k, v in inputs.items()}
    nc = _get_nc()
    in_maps = prep_in_maps(**inputs)
    res = run_bass_kernel_spmd(nc, in_maps, list(range(NCORES)))
    outT = np.stack([res.results[c]["out_T"] for c in range(NCORES)])  # bf16
    out = outT.astype(np.float32).transpose(0, 3, 1, 2).reshape(B, T, 2 * H)
    return np.ascontiguousarray(out)
